# revision 6
# baseline (speedup 1.0000x reference)
"""Dynamic 3x3 per-pixel filter (DynamicFilterLayer2D) on 8 Trainium2 cores.

Reference: out[b,c,h,w] = sum_{i,j in 3x3} xpad[b,c,h+i,w+j] * f[b,c,(3i+j),h,w]

Sharding: H is split into 8 bands of 32 rows; each core processes all
(b, c) images for its band (data parallel, 1-row halo). Per-core layout:
partitions = 128 (b,c) images (2 groups of 128), free dim = flat pixels.

Compute: a custom *segmented* DVE scan op streams [pixel, j-tap] pairs —
x via an overlapping access pattern, filters host-interleaved to
[..., w, j] — accumulating Src0*Src1 in fp32 and resetting the
accumulator at each 3-element segment boundary (a SUB_DIM_DONE "step"
uop state re-seeds with Zero + product). The output AP uses a stride-0
inner dim so only each pixel's segment-end value lands in SBUF:
T_i[p] = per-pixel 3-tap sum, compact bf16. Three such scans (one per
i row-tap) are combined with two bf16 tensor_tensor adds which run in
the DVE's 2x packed mode. Inputs and the output travel as bf16 (halves
HBM traffic; rel err ~5e-3 stays well inside the 2e-2 gate). Filter
border columns (taps that would multiply x-padding) are zeroed
host-side, so x tiles need no column padding and all access patterns
have uniform strides.
"""

import numpy as np

B, C, H, W = 8, 32, 256, 256
K = 3
N_CORES = 8
BAND = H // N_CORES            # 32 rows per core
RD = 8                         # rows per super-tile (one compute block)
N_SUPERS = BAND // RD          # 4
N_IMG = B * C                  # 256 images
P = 128
N_IMG_GROUPS = N_IMG // P      # 2
FD = RD * W                    # pixels per partition per super-tile (2048)
X_SUPER = FD + 2 * W + 2       # x elements per super-tile (guards incl)
X_FLAT = (BAND + 2) * W + 2    # per-image padded x row storage

_CACHE = {}


def _register_seg_mac_scan():
    """Custom DVE op: segmented multiply-accumulate scan.

    body = scan(ADD, Src0*Src1) over a [P, S, N] stream, with an extra
    'step' uop state entered on SUB_DIM_DONE that computes
    `Zero + Src0*Src1` at the scan stage — i.e. the running sum resets
    at every inner-dim (N) boundary. With N=K=3 each pixel's last
    element holds its private 3-tap dot product.
    """
    from concourse import dve_ops, dve_spec as ds
    from concourse.dve_ops import DveOp
    from concourse.dve_uop import DveOpSpec, Trigger

    name = "ANT_SEGMAC_SCAN"
    for op in dve_ops.OPS:
        if op.name == name:
            return op

    def _ref(in0, in1, s0, s1, imm2):
        p = np.asarray(in0, np.float32) * np.asarray(in1, np.float32)
        seg = p.reshape(p.shape[0], -1, K)
        return np.cumsum(seg, axis=2, dtype=np.float32).reshape(p.shape)

    spec = ds.Spec(body=ds.scan(ds.AluOp.ADD, ds.Src0 * ds.Src1),
                   reference=_ref)
    op = DveOp(name, spec, True, {})
    dve_ops.OPS.append(op)
    dve_ops.CUSTOM_DVE_SPECS[name] = spec
    dve_ops._SUB_OPCODE_FOR_NAME[name] = (
        dve_ops._CUSTOM_DVE_ROW_BASE + len(dve_ops.OPS) - 1)

    for ver in ("v3", "v4"):
        n_lanes, n_stages = ds.N_LANES[ver], ds.N_STAGES[ver]
        ds._validate_body(spec, ver)
        spec2 = ds._hoist_stream_invariant_ops(spec)
        scans = ds._collect(spec2.body, ds.Scan)
        latches = ds._collect(spec2.body, ds.Latch)
        placement = ds._build_placement(spec2, scans, n_stages, n_lanes)
        states = ds._build_state_machine(spec2, scans, latches, placement)
        # stock states: [seed, steady]. Rewire steady to branch to an
        # appended 'step' state on SUB_DIM_DONE; step consumes exactly one
        # element (the first of the new segment) with the scan stage
        # overridden to Zero + expr (accumulator reset), then returns.
        assert len(states) == 2, states
        steady_idx = 1
        step_idx = 2
        scan_node = scans[0]
        d = placement.node_stage[scan_node]
        steady = states[steady_idx]
        states[steady_idx] = ds._State(
            placement=placement,
            consume=steady.consume,
            trigger=(Trigger.SRC_TENSOR_DONE, Trigger.SUB_DIM_DONE,
                     Trigger.NONE),
            next=(0, step_idx, 0),
        )
        states.append(ds._State(
            placement=placement,
            consume=steady.consume,
            overrides={d: ds._Stage(ds.AluOp.ADD, ds.Zero, scan_node.expr)},
            trigger=(Trigger.SRC_TENSOR_DONE, Trigger.SUB_DIM_DONE,
                     Trigger.COUNT),
            next=(0, step_idx, steady_idx),
            repeat=1,
        ))
        uops = [ds._assemble(s) for s in states]
        for u in uops:
            u.validate(ver)
        dve_ops._COMPILE_CACHE[(name, ver)] = DveOpSpec(
            name=name,
            opcode=dve_ops.get_dve_sub_opcode(name),
            uops=uops,
            rd1_en=True,
        )
    return op


def _strided_ap(tile_ap, dims, offset):
    """Copy of tile_ap with free dims replaced by [[step, count], ...]
    (element units) at element offset; partition dim preserved."""
    import bass_rust
    c = tile_ap.copy()
    part = list(c.ap)[0]
    c.ap = bass_rust.VecI64Pair([list(part)] + [list(d) for d in dims])
    c.offset = offset
    return c


def _build_module():
    import concourse.bacc as bacc
    import concourse.mybir as mybir
    from concourse.tile import TileContext

    mac_scan = _register_seg_mac_scan()
    bf16 = mybir.dt.bfloat16
    add = mybir.AluOpType.add

    nc = bacc.Bacc("TRN2", target_bir_lowering=False, debug=False)
    x_d = nc.dram_tensor("x_s", [N_IMG, X_FLAT], bf16,
                         kind="ExternalInput").ap()
    # host-interleaved filters: [img, i, band_row, w, j]
    f_d = nc.dram_tensor("f_s", [N_IMG, K, BAND, W, K], bf16,
                         kind="ExternalInput").ap()
    o_d = nc.dram_tensor("o_s", [N_IMG, BAND, W], bf16,
                         kind="ExternalOutput").ap()

    with TileContext(nc) as tc:
        with (
            tc.tile_pool(name="xp", bufs=2) as xpool,
            tc.tile_pool(name="fp", bufs=6) as fpool,
            tc.tile_pool(name="s0p", bufs=2) as s0pool,
            tc.tile_pool(name="s1p", bufs=2) as s1pool,
            tc.tile_pool(name="s2p", bufs=2) as s2pool,
            tc.tile_pool(name="vp", bufs=2) as vpool,
            tc.tile_pool(name="op", bufs=2) as opool,
        ):
            scpools = [s0pool, s1pool, s2pool]
            # per image-group list of (row_start, rows) super-tiles; start
            # small so compute starts as soon as the first filter bytes
            # land, end small so the final compute backlog is short
            supers = {
                g: [(t2 * RD, RD) for t2 in range(N_SUPERS)]
                for g in range(N_IMG_GROUPS)
            }
            supers[0] = ([(0, 1), (1, 2), (3, 5)]
                         + [(t2 * RD, RD) for t2 in range(1, N_SUPERS)])
            supers[N_IMG_GROUPS - 1] = (
                [(t2 * RD, RD) for t2 in range(N_SUPERS - 1)]
                + [(BAND - RD, 4), (BAND - 4, 2), (BAND - 2, 1), (BAND - 1, 1)]
            )
            for g in range(N_IMG_GROUPS):
                for (r0, rd) in supers[g]:
                    p0 = g * P
                    fd = rd * W
                    fs = fd          # pixels in this super
                    xt = xpool.tile([P, X_SUPER], bf16, tag="x")
                    nc.scalar.dma_start(
                        out=xt[:, 0:fd + 2 * W + 2],
                        in_=x_d[p0:p0 + P, r0 * W: r0 * W + fd + 2 * W + 2],
                    )
                    fts = []
                    for i in range(K):
                        ft = fpool.tile([P, K * FD], bf16, tag="f", name="ft")
                        nc.sync.dma_start(
                            out=ft[:, 0:K * fd],
                            in_=f_d[p0:p0 + P, i, r0: r0 + rd, :, :],
                        )
                        fts.append(ft)
                    ot = opool.tile([P, FD], bf16, tag="o")
                    vt = vpool.tile([P, FD], bf16, tag="v", name="vt")
                    scs = []
                    for i in range(K):
                        sct = scpools[i].tile([P, FD], bf16,
                                              tag=f"sc{i}", name="sct")
                        in0 = _strided_ap(xt[:, :], [[1, fs], [1, K]], i * W)
                        in1 = _strided_ap(fts[i][:, :], [[K, fs], [1, K]], 0)
                        # stride-0 inner dim: 3 writes/pixel land on one
                        # slot, the last (the segment sum) wins -> compact
                        sc_out = _strided_ap(sct[:, :], [[1, fs], [0, K]], 0)
                        nc.vector._custom_dve(mac_scan, out=sc_out,
                                              in0=in0, in1=in1)
                        scs.append(sct)
                    # bf16 step-1 adds on the otherwise-idle GpSimd engine
                    # (light duty, so SBUF-port contention with the DVE
                    # scans stays small)
                    nc.gpsimd.tensor_tensor(
                        vt[:, 0:fs], scs[0][:, 0:fs], scs[1][:, 0:fs], add)
                    nc.gpsimd.tensor_tensor(
                        ot[:, 0:fs], vt[:, 0:fs], scs[2][:, 0:fs], add)
                    nc.gpsimd.dma_start(
                        out=o_d[p0:p0 + P, r0:r0 + rd, :],
                        in_=ot[:, 0:fd],
                    )
    nc.compile()
    return nc


def _get_module():
    if "nc" not in _CACHE:
        _CACHE["nc"] = _build_module()
    return _CACHE["nc"]


def _shard_inputs(x, dynamic_filters):
    """Per-core input maps. x: [B,C,H,W] f32, filters: [B,C*9,H,W] f32."""
    import ml_dtypes
    bf16 = ml_dtypes.bfloat16

    xb = x.astype(bf16)
    xp = np.pad(xb, ((0, 0), (0, 0), (1, 1), (0, 0)))  # pad rows only
    # filters -> [B, C, i, j, H, W] -> zero border cols -> [img, i, H, W, j]
    f6 = dynamic_filters.astype(bf16).reshape(B, C, K, K, H, W)
    f6[:, :, :, 0, :, 0] = 0.0      # j=0 taps multiply x col -1
    f6[:, :, :, 2, :, W - 1] = 0.0  # j=2 taps multiply x col W
    f_int = np.ascontiguousarray(
        f6.transpose(0, 1, 2, 4, 5, 3)).reshape(N_IMG, K, H, W, K)

    in_maps = []
    for n in range(N_CORES):
        r = n * BAND
        xs = xp[:, :, r:r + BAND + 2, :].reshape(N_IMG, (BAND + 2) * W)
        xs_flat = np.zeros((N_IMG, X_FLAT), bf16)
        xs_flat[:, 1:-1] = xs
        fs = np.ascontiguousarray(f_int[:, :, r:r + BAND])
        in_maps.append({"x_s": xs_flat, "f_s": fs})
    return in_maps


def kernel(x, dynamic_filters, _trace=False):
    from concourse import bass_utils

    x = np.asarray(x, dtype=np.float32)
    dynamic_filters = np.asarray(dynamic_filters, dtype=np.float32)
    nc = _get_module()
    in_maps = _shard_inputs(x, dynamic_filters)
    res = bass_utils.run_bass_kernel_spmd(
        nc, in_maps, list(range(N_CORES)), trace=_trace)
    out = np.concatenate(
        [np.asarray(res.results[n]["o_s"]).astype(np.float32)
         .reshape(B, C, BAND, W) for n in range(N_CORES)],
        axis=2)
    _CACHE["last_exec_time_ns"] = res.exec_time_ns
    return out


# revision 7
# speedup vs baseline: 1.2071x; 1.2071x over previous
"""Dynamic 3x3 per-pixel filter (DynamicFilterLayer2D) on 8 Trainium2 cores.

Reference: out[b,c,h,w] = sum_{i,j in 3x3} xpad[b,c,h+i,w+j] * f[b,c,(3i+j),h,w]

Sharding: H is split into 8 bands of 32 rows; each core processes all
(b, c) images for its band (data parallel, 1-row halo). Per-core layout:
partitions = 128 (b,c) images (2 groups of 128), free dim = flat pixels.

Compute: a custom *segmented* DVE scan op streams [pixel, j-tap] pairs —
x via an overlapping access pattern, filters host-interleaved to
[..., w, j] — accumulating Src0*Src1 in fp32 and resetting the
accumulator at each 3-element segment boundary (a SUB_DIM_DONE "step"
uop state re-seeds with Zero + product). The output AP uses a stride-0
inner dim so only each pixel's segment-end value lands in SBUF:
T_i[p] = per-pixel 3-tap sum, compact bf16. Three such scans (one per
i row-tap) are combined with two bf16 tensor_tensor adds which run in
the DVE's 2x packed mode. Inputs and the output travel as bf16 (halves
HBM traffic; rel err ~5e-3 stays well inside the 2e-2 gate). Filter
border columns (taps that would multiply x-padding) are zeroed
host-side, so x tiles need no column padding and all access patterns
have uniform strides.
"""

import numpy as np

B, C, H, W = 8, 32, 256, 256
K = 3
N_CORES = 8
BAND = H // N_CORES            # 32 rows per core
RD = 8                         # rows per super-tile (one compute block)
N_SUPERS = BAND // RD          # 4
N_IMG = B * C                  # 256 images
P = 128
N_IMG_GROUPS = N_IMG // P      # 2
FD = RD * W                    # pixels per partition per super-tile (2048)
X_SUPER = FD + 2 * W + 2       # x elements per super-tile (guards incl)
X_FLAT = (BAND + 2) * W + 2    # per-image padded x row storage

_CACHE = {}


def _register_seg_mac_scan():
    """Custom DVE op: segmented multiply-accumulate scan.

    body = scan(ADD, Src0*Src1) over a [P, S, N] stream, with an extra
    'step' uop state entered on SUB_DIM_DONE that computes
    `Zero + Src0*Src1` at the scan stage — i.e. the running sum resets
    at every inner-dim (N) boundary. With N=K=3 each pixel's last
    element holds its private 3-tap dot product.
    """
    from concourse import dve_ops, dve_spec as ds
    from concourse.dve_ops import DveOp
    from concourse.dve_uop import DveOpSpec, Trigger

    name = "ANT_SEGMAC_SCAN"
    for op in dve_ops.OPS:
        if op.name == name:
            return op

    def _ref(in0, in1, s0, s1, imm2):
        p = np.asarray(in0, np.float32) * np.asarray(in1, np.float32)
        seg = p.reshape(p.shape[0], -1, K)
        return np.cumsum(seg, axis=2, dtype=np.float32).reshape(p.shape)

    spec = ds.Spec(body=ds.scan(ds.AluOp.ADD, ds.Src0 * ds.Src1),
                   reference=_ref)
    op = DveOp(name, spec, True, {})
    dve_ops.OPS.append(op)
    dve_ops.CUSTOM_DVE_SPECS[name] = spec
    dve_ops._SUB_OPCODE_FOR_NAME[name] = (
        dve_ops._CUSTOM_DVE_ROW_BASE + len(dve_ops.OPS) - 1)

    for ver in ("v3", "v4"):
        n_lanes, n_stages = ds.N_LANES[ver], ds.N_STAGES[ver]
        ds._validate_body(spec, ver)
        spec2 = ds._hoist_stream_invariant_ops(spec)
        scans = ds._collect(spec2.body, ds.Scan)
        latches = ds._collect(spec2.body, ds.Latch)
        placement = ds._build_placement(spec2, scans, n_stages, n_lanes)
        states = ds._build_state_machine(spec2, scans, latches, placement)
        # stock states: [seed, steady]. Rewire steady to branch to an
        # appended 'step' state on SUB_DIM_DONE; step consumes exactly one
        # element (the first of the new segment) with the scan stage
        # overridden to Zero + expr (accumulator reset), then returns.
        assert len(states) == 2, states
        steady_idx = 1
        step_idx = 2
        scan_node = scans[0]
        d = placement.node_stage[scan_node]
        steady = states[steady_idx]
        states[steady_idx] = ds._State(
            placement=placement,
            consume=steady.consume,
            trigger=(Trigger.SRC_TENSOR_DONE, Trigger.SUB_DIM_DONE,
                     Trigger.NONE),
            next=(0, step_idx, 0),
        )
        states.append(ds._State(
            placement=placement,
            consume=steady.consume,
            overrides={d: ds._Stage(ds.AluOp.ADD, ds.Zero, scan_node.expr)},
            trigger=(Trigger.SRC_TENSOR_DONE, Trigger.SUB_DIM_DONE,
                     Trigger.COUNT),
            next=(0, step_idx, steady_idx),
            repeat=1,
        ))
        uops = [ds._assemble(s) for s in states]
        for u in uops:
            u.validate(ver)
        dve_ops._COMPILE_CACHE[(name, ver)] = DveOpSpec(
            name=name,
            opcode=dve_ops.get_dve_sub_opcode(name),
            uops=uops,
            rd1_en=True,
        )
    return op


def _strided_ap(tile_ap, dims, offset):
    """Copy of tile_ap with free dims replaced by [[step, count], ...]
    (element units) at element offset; partition dim preserved."""
    import bass_rust
    c = tile_ap.copy()
    part = list(c.ap)[0]
    c.ap = bass_rust.VecI64Pair([list(part)] + [list(d) for d in dims])
    c.offset = offset
    return c


def _build_module():
    import concourse.bacc as bacc
    import concourse.mybir as mybir
    from concourse.tile import TileContext

    mac_scan = _register_seg_mac_scan()
    bf16 = mybir.dt.bfloat16
    add = mybir.AluOpType.add

    nc = bacc.Bacc("TRN2", target_bir_lowering=False, debug=False)
    x_d = nc.dram_tensor("x_s", [N_IMG, X_FLAT], bf16,
                         kind="ExternalInput").ap()
    # host-interleaved filters: [img, i, band_row, w, j]
    f_d = nc.dram_tensor("f_s", [N_IMG, K, BAND, W, K], bf16,
                         kind="ExternalInput").ap()
    o_d = nc.dram_tensor("o_s", [N_IMG, BAND, W], bf16,
                         kind="ExternalOutput").ap()

    with TileContext(nc) as tc:
        with (
            tc.tile_pool(name="xp", bufs=2) as xpool,
            tc.tile_pool(name="fp", bufs=6) as fpool,
            tc.tile_pool(name="s0p", bufs=2) as s0pool,
            tc.tile_pool(name="s1p", bufs=2) as s1pool,
            tc.tile_pool(name="s2p", bufs=2) as s2pool,
            tc.tile_pool(name="vp", bufs=2) as vpool,
            tc.tile_pool(name="op", bufs=2) as opool,
        ):
            scpools = [s0pool, s1pool, s2pool]
            # per image-group list of (row_start, rows) super-tiles; start
            # small so compute starts as soon as the first filter bytes
            # land, end small so the final compute backlog is short
            supers = {
                g: [(t2 * RD, RD) for t2 in range(N_SUPERS)]
                for g in range(N_IMG_GROUPS)
            }
            supers[0] = ([(0, 1), (1, 2), (3, 5)]
                         + [(t2 * RD, RD) for t2 in range(1, N_SUPERS)])
            supers[N_IMG_GROUPS - 1] = (
                [(t2 * RD, RD) for t2 in range(N_SUPERS - 1)]
                + [(BAND - RD, 4), (BAND - 4, 2), (BAND - 2, 1), (BAND - 1, 1)]
            )
            for g in range(N_IMG_GROUPS):
                for (r0, rd) in supers[g]:
                    p0 = g * P
                    fd = rd * W
                    fs = fd          # pixels in this super
                    xt = xpool.tile([P, X_SUPER], bf16, tag="x")
                    nc.scalar.dma_start(
                        out=xt[:, 0:fd + 2 * W + 2],
                        in_=x_d[p0:p0 + P, r0 * W: r0 * W + fd + 2 * W + 2],
                    )
                    fts = []
                    for i in range(K):
                        ft = fpool.tile([P, K * FD], bf16, tag="f", name="ft")
                        nc.sync.dma_start(
                            out=ft[:, 0:K * fd],
                            in_=f_d[p0:p0 + P, i, r0: r0 + rd, :, :],
                        )
                        fts.append(ft)
                    ot = opool.tile([P, FD], bf16, tag="o")
                    vt = vpool.tile([P, FD], bf16, tag="v", name="vt")
                    scs = []
                    for i in range(K):
                        sct = scpools[i].tile([P, FD], bf16,
                                              tag=f"sc{i}", name="sct")
                        in0 = _strided_ap(xt[:, :], [[1, fs], [1, K]], i * W)
                        in1 = _strided_ap(fts[i][:, :], [[K, fs], [1, K]], 0)
                        # stride-0 inner dim: 3 writes/pixel land on one
                        # slot, the last (the segment sum) wins -> compact
                        sc_out = _strided_ap(sct[:, :], [[1, fs], [0, K]], 0)
                        nc.vector._custom_dve(mac_scan, out=sc_out,
                                              in0=in0, in1=in1)
                        scs.append(sct)
                    # bf16 step-1 adds: DVE 2x packed mode
                    nc.vector.tensor_tensor(
                        vt[:, 0:fs], scs[0][:, 0:fs], scs[1][:, 0:fs], add)
                    nc.vector.tensor_tensor(
                        ot[:, 0:fs], vt[:, 0:fs], scs[2][:, 0:fs], add)
                    nc.gpsimd.dma_start(
                        out=o_d[p0:p0 + P, r0:r0 + rd, :],
                        in_=ot[:, 0:fd],
                    )
    nc.compile()
    return nc


def _get_module():
    if "nc" not in _CACHE:
        _CACHE["nc"] = _build_module()
    return _CACHE["nc"]


def _shard_inputs(x, dynamic_filters):
    """Per-core input maps. x: [B,C,H,W] f32, filters: [B,C*9,H,W] f32."""
    import ml_dtypes
    bf16 = ml_dtypes.bfloat16

    xb = x.astype(bf16)
    xp = np.pad(xb, ((0, 0), (0, 0), (1, 1), (0, 0)))  # pad rows only
    # filters -> [B, C, i, j, H, W] -> zero border cols -> [img, i, H, W, j]
    f6 = dynamic_filters.astype(bf16).reshape(B, C, K, K, H, W)
    f6[:, :, :, 0, :, 0] = 0.0      # j=0 taps multiply x col -1
    f6[:, :, :, 2, :, W - 1] = 0.0  # j=2 taps multiply x col W
    f_int = np.ascontiguousarray(
        f6.transpose(0, 1, 2, 4, 5, 3)).reshape(N_IMG, K, H, W, K)

    in_maps = []
    for n in range(N_CORES):
        r = n * BAND
        xs = xp[:, :, r:r + BAND + 2, :].reshape(N_IMG, (BAND + 2) * W)
        xs_flat = np.zeros((N_IMG, X_FLAT), bf16)
        xs_flat[:, 1:-1] = xs
        fs = np.ascontiguousarray(f_int[:, :, r:r + BAND])
        in_maps.append({"x_s": xs_flat, "f_s": fs})
    return in_maps


def kernel(x, dynamic_filters, _trace=False):
    from concourse import bass_utils

    x = np.asarray(x, dtype=np.float32)
    dynamic_filters = np.asarray(dynamic_filters, dtype=np.float32)
    nc = _get_module()
    in_maps = _shard_inputs(x, dynamic_filters)
    res = bass_utils.run_bass_kernel_spmd(
        nc, in_maps, list(range(N_CORES)), trace=_trace)
    out = np.concatenate(
        [np.asarray(res.results[n]["o_s"]).astype(np.float32)
         .reshape(B, C, BAND, W) for n in range(N_CORES)],
        axis=2)
    _CACHE["last_exec_time_ns"] = res.exec_time_ns
    return out


# revision 8
# speedup vs baseline: 1.2765x; 1.0575x over previous
"""Dynamic 3x3 per-pixel filter (DynamicFilterLayer2D) on 8 Trainium2 cores.

Reference: out[b,c,h,w] = sum_{i,j in 3x3} xpad[b,c,h+i,w+j] * f[b,c,(3i+j),h,w]

Sharding: H is split into 8 bands of 32 rows; each core processes all
(b, c) images for its band (data parallel, 1-row halo). Per-core layout:
partitions = 128 (b,c) images (2 groups of 128), free dim = flat pixels.

Compute: x is host-interleaved 3-wide, x3[3*(r*W + w) + i] =
xpad[r-1+i, w], so the 9 taps of pixel g=(r,w) — xpad[r-1+i, w-1+j] =
x3[3g + 3j + i] — form 9 *consecutive* elements (tap order (j,i) is
fine: addition commutes). Filters are host-interleaved to the matching
[..., w, t=3j+i] order. A custom *segmented* DVE scan op streams
[pixel, tap] pairs, accumulating Src0*Src1 in fp32 and resetting the
accumulator at each 9-element segment boundary (a SUB_DIM_DONE "step"
uop state re-seeds with Zero + product). The output AP uses a stride-0
inner dim so only each pixel's segment-end value — the full 3x3 dot
product — lands in SBUF, written compactly straight into the bf16
output tile. One DVE instruction per super-tile; no other compute.

Everything travels as bf16 (rel err ~3e-3, well inside the 2e-2 gate);
accumulation stays fp32 inside the DVE. Filter border columns (taps
that would multiply x-padding) are zeroed host-side, so all access
patterns have uniform strides.
"""

import numpy as np

B, C, H, W = 8, 32, 256, 256
K = 3
KK = K * K
N_CORES = 8
BAND = H // N_CORES            # 32 rows per core
RD = 8                         # rows per super-tile (one compute block)
N_SUPERS = BAND // RD          # 4
N_IMG = B * C                  # 256 images
P = 128
N_IMG_GROUPS = N_IMG // P      # 2
FD = RD * W                    # pixels per partition per super-tile (2048)
X3_SUPER = 3 * FD + 6          # x3 elements per super-tile (guards incl)
X3_FLAT = 3 * BAND * W + 6     # per-image interleaved x storage

_CACHE = {}


def _register_seg_mac_scan():
    """Custom DVE op: segmented multiply-accumulate scan.

    body = scan(ADD, Src0*Src1) over a [P, S, N] stream, with an extra
    'step' uop state entered on SUB_DIM_DONE that computes
    `Zero + Src0*Src1` at the scan stage — i.e. the running sum resets
    at every inner-dim (N) boundary. With N=9 each pixel's last element
    holds its private 3x3 dot product.
    """
    from concourse import dve_ops, dve_spec as ds
    from concourse.dve_ops import DveOp
    from concourse.dve_uop import DveOpSpec, Trigger

    name = "ANT_SEGMAC_SCAN"
    for op in dve_ops.OPS:
        if op.name == name:
            return op

    def _ref(in0, in1, s0, s1, imm2):
        p = np.asarray(in0, np.float32) * np.asarray(in1, np.float32)
        seg = p.reshape(p.shape[0], -1, p.shape[-1])
        return np.cumsum(seg, axis=2, dtype=np.float32).reshape(p.shape)

    spec = ds.Spec(body=ds.scan(ds.AluOp.ADD, ds.Src0 * ds.Src1),
                   reference=_ref)
    op = DveOp(name, spec, True, {})
    dve_ops.OPS.append(op)
    dve_ops.CUSTOM_DVE_SPECS[name] = spec
    dve_ops._SUB_OPCODE_FOR_NAME[name] = (
        dve_ops._CUSTOM_DVE_ROW_BASE + len(dve_ops.OPS) - 1)

    for ver in ("v3", "v4"):
        n_lanes, n_stages = ds.N_LANES[ver], ds.N_STAGES[ver]
        ds._validate_body(spec, ver)
        spec2 = ds._hoist_stream_invariant_ops(spec)
        scans = ds._collect(spec2.body, ds.Scan)
        latches = ds._collect(spec2.body, ds.Latch)
        placement = ds._build_placement(spec2, scans, n_stages, n_lanes)
        states = ds._build_state_machine(spec2, scans, latches, placement)
        # stock states: [seed, steady]. Rewire steady to branch to an
        # appended 'step' state on SUB_DIM_DONE; step consumes exactly one
        # element (the first of the new segment) with the scan stage
        # overridden to Zero + expr (accumulator reset), then returns.
        assert len(states) == 2, states
        steady_idx = 1
        step_idx = 2
        scan_node = scans[0]
        d = placement.node_stage[scan_node]
        steady = states[steady_idx]
        states[steady_idx] = ds._State(
            placement=placement,
            consume=steady.consume,
            trigger=(Trigger.SRC_TENSOR_DONE, Trigger.SUB_DIM_DONE,
                     Trigger.NONE),
            next=(0, step_idx, 0),
        )
        states.append(ds._State(
            placement=placement,
            consume=steady.consume,
            overrides={d: ds._Stage(ds.AluOp.ADD, ds.Zero, scan_node.expr)},
            trigger=(Trigger.SRC_TENSOR_DONE, Trigger.SUB_DIM_DONE,
                     Trigger.COUNT),
            next=(0, step_idx, steady_idx),
            repeat=1,
        ))
        uops = [ds._assemble(s) for s in states]
        for u in uops:
            u.validate(ver)
        dve_ops._COMPILE_CACHE[(name, ver)] = DveOpSpec(
            name=name,
            opcode=dve_ops.get_dve_sub_opcode(name),
            uops=uops,
            rd1_en=True,
        )
    return op


def _strided_ap(tile_ap, dims, offset):
    """Copy of tile_ap with free dims replaced by [[step, count], ...]
    (element units) at element offset; partition dim preserved."""
    import bass_rust
    c = tile_ap.copy()
    part = list(c.ap)[0]
    c.ap = bass_rust.VecI64Pair([list(part)] + [list(d) for d in dims])
    c.offset = offset
    return c


def _build_module():
    import concourse.bacc as bacc
    import concourse.mybir as mybir
    from concourse.tile import TileContext

    mac_scan = _register_seg_mac_scan()
    bf16 = mybir.dt.bfloat16

    nc = bacc.Bacc("TRN2", target_bir_lowering=False, debug=False)
    x_d = nc.dram_tensor("x_s", [N_IMG, X3_FLAT], bf16,
                         kind="ExternalInput").ap()
    # host-interleaved filters: [img, band_row, w, t] with t = 3j + i
    f_d = nc.dram_tensor("f_s", [N_IMG, BAND, W, KK], bf16,
                         kind="ExternalInput").ap()
    o_d = nc.dram_tensor("o_s", [N_IMG, BAND, W], bf16,
                         kind="ExternalOutput").ap()

    with TileContext(nc) as tc:
        with (
            tc.tile_pool(name="xp", bufs=2) as xpool,
            tc.tile_pool(name="fp", bufs=2) as fpool,
            tc.tile_pool(name="op", bufs=2) as opool,
        ):
            # per image-group list of (row_start, rows) super-tiles; start
            # small so compute starts as soon as the first filter bytes
            # land, end small so the final compute backlog is short
            supers = {
                g: [(t2 * RD, RD) for t2 in range(N_SUPERS)]
                for g in range(N_IMG_GROUPS)
            }
            supers[0] = ([(0, 2), (2, 6)]
                         + [(t2 * RD, RD) for t2 in range(1, N_SUPERS)])
            supers[N_IMG_GROUPS - 1] = (
                [(t2 * RD, RD) for t2 in range(N_SUPERS - 1)]
                + [(BAND - RD, 4), (BAND - 4, 2), (BAND - 2, 1), (BAND - 1, 1)]
            )
            for g in range(N_IMG_GROUPS):
                for (r0, rd) in supers[g]:
                    p0 = g * P
                    fd = rd * W
                    fs = fd          # pixels in this super
                    xt = xpool.tile([P, X3_SUPER], bf16, tag="x")
                    nc.scalar.dma_start(
                        out=xt[:, 0:3 * fd + 6],
                        in_=x_d[p0:p0 + P,
                                3 * r0 * W: 3 * (r0 * W + fd) + 6],
                    )
                    ft = fpool.tile([P, KK * FD], bf16, tag="f", name="ft")
                    nc.sync.dma_start(
                        out=ft[:, 0:KK * fd],
                        in_=f_d[p0:p0 + P, r0: r0 + rd, :, :],
                    )
                    ot = opool.tile([P, FD], bf16, tag="o")
                    in0 = _strided_ap(xt[:, :], [[3, fs], [1, KK]], 0)
                    in1 = _strided_ap(ft[:, :], [[KK, fs], [1, KK]], 0)
                    # stride-0 inner dim: 9 writes/pixel land on one slot,
                    # the last (the full 3x3 dot product) wins -> compact
                    sc_out = _strided_ap(ot[:, :], [[1, fs], [0, KK]], 0)
                    nc.vector._custom_dve(mac_scan, out=sc_out,
                                          in0=in0, in1=in1)
                    nc.gpsimd.dma_start(
                        out=o_d[p0:p0 + P, r0:r0 + rd, :],
                        in_=ot[:, 0:fd],
                    )
    nc.compile()
    return nc


def _get_module():
    if "nc" not in _CACHE:
        _CACHE["nc"] = _build_module()
    return _CACHE["nc"]


def _shard_inputs(x, dynamic_filters):
    """Per-core input maps. x: [B,C,H,W] f32, filters: [B,C*9,H,W] f32."""
    import ml_dtypes
    bf16 = ml_dtypes.bfloat16

    xb = x.astype(bf16)
    xp = np.pad(xb, ((0, 0), (0, 0), (1, 1), (0, 0)))  # pad rows only
    # x3[b,c,r,w,i] = xpad[b,c,r+i,w]  (r+i spans r-1..r+1 pre-pad)
    x3 = np.stack((xp[:, :, 0:H], xp[:, :, 1:H + 1], xp[:, :, 2:H + 2]),
                  axis=-1)
    # filters -> [B, C, i, j, H, W] -> zero border cols
    #          -> [img, H, W, t=3j+i]
    f6 = dynamic_filters.astype(bf16).reshape(B, C, K, K, H, W)
    f6[:, :, :, 0, :, 0] = 0.0      # j=0 taps multiply x col -1
    f6[:, :, :, 2, :, W - 1] = 0.0  # j=2 taps multiply x col W
    f_int = np.ascontiguousarray(
        f6.transpose(0, 1, 4, 5, 3, 2)).reshape(N_IMG, H, W, KK)

    in_maps = []
    for n in range(N_CORES):
        r = n * BAND
        xs = x3[:, :, r:r + BAND].reshape(N_IMG, 3 * BAND * W)
        xs_flat = np.zeros((N_IMG, X3_FLAT), bf16)
        xs_flat[:, 3:-3] = xs
        fs = np.ascontiguousarray(f_int[:, r:r + BAND])
        in_maps.append({"x_s": xs_flat, "f_s": fs})
    return in_maps


def kernel(x, dynamic_filters, _trace=False):
    from concourse import bass_utils

    x = np.asarray(x, dtype=np.float32)
    dynamic_filters = np.asarray(dynamic_filters, dtype=np.float32)
    nc = _get_module()
    in_maps = _shard_inputs(x, dynamic_filters)
    res = bass_utils.run_bass_kernel_spmd(
        nc, in_maps, list(range(N_CORES)), trace=_trace)
    out = np.concatenate(
        [np.asarray(res.results[n]["o_s"]).astype(np.float32)
         .reshape(B, C, BAND, W) for n in range(N_CORES)],
        axis=2)
    _CACHE["last_exec_time_ns"] = res.exec_time_ns
    return out
